# revision 49
# baseline (speedup 1.0000x reference)
"""Trainium2 Bass kernel v4: fused single-pass fp16 viscous-RHS.

v4 over v3 (200.6us -> 160.1us):
- 9-slot symmetric flux layout [t00 t01 t11 t02 ez t12 ey t22 ex]
  exploits tau symmetry: the 3 Pool tensor-copies (t10,t02,t12 dups)
  are gone; stage-C streams read shared slots, x-divergence of the
  momentum pass uses two single-field matmuls.
- diag-mu derived on chip (md = 2*m2 via one 4x-mode TensorScalar)
  instead of pre-doubling the three diagonal derivative slots.
- B2 product block regrouped per-u_i so all 9 energy accumulate-adds
  are three stride-2 instructions; ez edge-planes fall out of the
  batched full-E products.
- dy drains go to dedicated dym/dyt tiles and B1 is split so all its
  dym reads happen first; next-chunk dy matmuls are emitted right
  after that point and fill PE during B1/B2.
- main dym chunk-overlap planes (locals 0,1 = prev locals ZC,ZC+1)
  are copied via a small staging tile (two-hop dodges the same-tag
  alloc deadlock) instead of redoing their dy matmuls + drains.
- main pre01 (= dy u0 + dz u1) assembled in the dy-PSUM itself: two
  extra +/-identity matmuls per plane fold the dz u1 term in, killing
  the Pool t01-add and the main dz-u1 DVE sub. (The symmetric t12
  absorb measured WORSE - PE is the binding engine after pre01.)
- first-chunk loads spread across SP/Act/Pool DMA queues; weight
  loads deferred behind them and spread by first-use; output stores
  ride the Act queue so they never serialize behind input prefetch.
- last chunk's B2/C1 split in two z-halves so the final energy-pass
  matmuls overlap the tail of the product block instead of running
  after it.
- DVE/Pool op assignment rebalanced (Pool/DVE ~80%, PE ~76% busy).
  Measured engine rates (CoreSim, fp16): DVE tensor-tensor 0.52
  ns/elem (2x mode), DVE tensor-scalar/copy 0.26 (4x), Pool ~0.83,
  Act ~0.85 (unary only), PE 0.42 ns/col/stream; PSUM ops drop DVE
  to 1.04. Act offload of the dv/mo scalar passes was tried and is
  a net LOSS (critical-path latency + 3x worse rate than DVE-4x).

Design vs v2 (two-pass fp32):
- Single pass: fluxes (tau rows + energy flux) are built in SBUF and the
  divergence is accumulated directly into PSUM by the PE via identity
  matmuls (dz/dx terms) + Dy matmuls (dy term). No DRAM round-trip.
- fp16 everywhere on-chip (tolerance is 2e-2; fp16 lands ~1e-3), which
  doubles DVE throughput and halves DMA bytes.
- Host pre-stages inputs already transposed to [y, field, z, x] fp16 with
  x-halo, so every DMA is large and fully contiguous (no transpose DMAs,
  ~6 DMA instructions per chunk instead of ~100).
- mu = MU_REF*(N/2)^2*T^0.7 is precomputed on host (kills ln/exp + act
  table loads). Scale folding: host stages u' = u/2, Tq = (CP/PR/4)*T,
  m2 = 2*mu; then tau_ij = m2*(du'-forms), e' = m2*dTq + sum tau*u' = e/2
  and the host doubles the energy output.
- y=192 rows are folded as [0:128) main tiles + 64-row tail folded into
  128 partitions as two z-half-ranges (2-plane overlap so z-shifts stay
  uniform per half).
"""

import sys

sys.path.insert(0, "/opt/trn_rl_repo")

import numpy as np

N = 192
NCORES = 8
NZ = N // NCORES          # 24 planes per core
ZC = 6                    # chunk size (output planes per chunk)
HC = ZC // 2
W = 194                   # flux x-width  (x in [-1, 193))
WI = 196                  # input x-width (x in [-2, 194))

MU_REF = 1.8e-5
PR = 0.72
CP = 1005.0
CPR = CP / PR
MUSCALE = MU_REF * (N / 2.0) ** 2   # both divergence scales folded into mu


def build_program(nz=NZ, num_devices=NCORES):
    import concourse.bacc as bacc
    import concourse.mybir as mybir
    from concourse.tile import TileContext

    f16 = mybir.dt.float16
    f32 = mybir.dt.float32
    assert nz % ZC == 0
    nchunks = nz // ZC
    nzp = nz + 4

    nc = bacc.Bacc("TRN2", target_bir_lowering=False, debug=False,
                   num_devices=num_devices)

    amain = nc.dram_tensor("amain", [128, 5, nzp, WI], f16,
                           kind="ExternalInput")
    atail = nc.dram_tensor("atail", [64, 5, nzp, WI], f16,
                           kind="ExternalInput")
    wts = nc.dram_tensor("wts", [192, 576], f16, kind="ExternalInput")
    omain = nc.dram_tensor("omain", [128, 4, nz, N], f16,
                           kind="ExternalOutput")
    otail = nc.dram_tensor("otail", [64, 4, nz, N], f16,
                           kind="ExternalOutput")

    E = ZC + 2        # extended plane count (main)
    ET = HC + 2       # extended planes per tail half (with overlap)
    IT = HC + 4       # input planes per tail half

    with TileContext(nc) as tc:
        with (
            tc.tile_pool(name="wpool", bufs=1) as wpool,
            tc.tile_pool(name="io", bufs=2) as io,
            tc.tile_pool(name="work", bufs=1) as wk,
            tc.tile_pool(name="psA", bufs=2, space="PSUM") as psA,
            tc.tile_pool(name="psC", bufs=4, space="PSUM") as psC,
        ):
            wap = wts.ap()
            wk0m0 = wpool.tile([128, 128], f16, tag="wk0m0")
            wk0m1 = wpool.tile([128, 64], f16, tag="wk0m1")
            wk1m0b0 = wpool.tile([64, 128], f16, tag="wk1m0b0")
            wk1m0b64 = wpool.tile([128, 128], f16, tag="wk1m0b64")
            wk1m1b0 = wpool.tile([64, 64], f16, tag="wk1m1b0")
            wk1m1b64 = wpool.tile([128, 64], f16, tag="wk1m1b64")
            wi = wpool.tile([128, 128], f16, tag="wi")
            wim = wpool.tile([128, 128], f16, tag="wim")
            wk1m1bd = wpool.tile([128, 128], f16, tag="wk1m1bd")

            def load_weights():
                # emitted AFTER the first input loads; spread across queues
                # by when each weight is first needed (main-dy first, C last)
                nc.sync.dma_start(out=wk0m0[:, :], in_=wap[0:128, 0:128])
                nc.sync.dma_start(out=wk1m0b0[:, :], in_=wap[128:192, 0:128])
                nc.sync.dma_start(out=wk1m0b64[64:128, :],
                                  in_=wap[128:192, 0:128])
                nc.scalar.dma_start(out=wk0m1[:, :], in_=wap[0:128, 128:192])
                nc.scalar.dma_start(out=wk1m1b0[:, :],
                                    in_=wap[128:192, 128:192])
                nc.scalar.dma_start(out=wk1m1b64[64:128, :],
                                    in_=wap[128:192, 128:192])
                nc.gpsimd.dma_start(out=wi[:, :], in_=wap[0:128, 192:320])
                nc.gpsimd.dma_start(out=wim[:, :], in_=wap[0:128, 320:448])
                nc.scalar.dma_start(out=wk1m1bd[:, :],
                                    in_=wap[0:128, 448:576])

            TT = nc.vector
            TP = nc.gpsimd

            def stage_load(c, first=False):
                z0 = c * ZC
                # first chunk: spread the three loads across SP/Act/DVE
                # DMA queues so startup isn't serialized on SP.
                e1 = nc.scalar if first else nc.sync
                e2 = nc.gpsimd if first else nc.sync
                inm_t = io.tile([128, 5 * (E + 2) * WI], f16, tag="inm")
                inm = inm_t.rearrange("p (f z x) -> p f z x", f=5, z=E + 2)
                if first:
                    h = (E + 2) // 2
                    nc.sync.dma_start(out=inm[:, :, 0:h, :],
                                      in_=amain.ap()[:, :, z0:z0 + h, :])
                    nc.scalar.dma_start(out=inm[:, :, h:E + 2, :],
                                        in_=amain.ap()[:, :, z0 + h:
                                                       z0 + E + 2, :])
                else:
                    nc.sync.dma_start(out=inm[:, :, :, :],
                                      in_=amain.ap()[:, :, z0:z0 + E + 2, :])
                int_t = io.tile([128, 5 * IT * WI], f16, tag="int")
                inT = int_t.rearrange("p (f z x) -> p f z x", f=5, z=IT)
                e1.dma_start(out=inT[0:64, :, :, :],
                             in_=atail.ap()[:, :, z0:z0 + IT, :])
                e2.dma_start(out=inT[64:128, :, :, :],
                             in_=atail.ap()[:, :, z0 + HC:z0 + HC + IT,
                                            :])
                s = {"z0": z0, "inm": inm, "inT": inT}
                return s

            def stage_A_sub(s):
                # z/x derivatives on DVE. D slots: 0..3 dz, 4..7 dx
                # (dy goes to the separate dym/dyt tiles via stage_A_dy).
                inm, inT = s["inm"], s["inT"]
                dm_t = wk.tile([128, 9 * E * W], f16, tag="dm")
                dm = dm_t.rearrange("p (s z x) -> p s z x", s=9, z=E)
                dt_t = wk.tile([128, 9 * ET * W], f16, tag="dt")
                dt = dt_t.rearrange("p (s z x) -> p s z x", s=9, z=ET)
                s["dm"], s["dt"] = dm, dt
                TT.tensor_sub(dm[:, 0:1, :, :],
                              inm[:, 0:1, 2:2 + E, 1:1 + W],
                              inm[:, 0:1, 0:E, 1:1 + W])
                TT.tensor_sub(dm[:, 2:4, :, :],
                              inm[:, 2:4, 2:2 + E, 1:1 + W],
                              inm[:, 2:4, 0:E, 1:1 + W])
                TT.tensor_sub(dt[:, 0:4, :, :],
                              inT[:, 0:4, 2:2 + ET, 1:1 + W],
                              inT[:, 0:4, 0:ET, 1:1 + W])
                TT.tensor_sub(dm[:, 4:8:2, :, :],
                              inm[:, 0:3:2, 1:1 + E, 2:2 + W],
                              inm[:, 0:3:2, 1:1 + E, 0:W])
                TT.tensor_sub(dm[:, 5:8:2, 1:E - 1, :],
                              inm[:, 1:4:2, 2:E, 2:2 + W],
                              inm[:, 1:4:2, 2:E, 0:W])
                TT.tensor_sub(dt[:, 4:8:2, :, :],
                              inT[:, 0:3:2, 1:1 + ET, 2:2 + W],
                              inT[:, 0:3:2, 1:1 + ET, 0:W])
                TT.tensor_sub(dt[:, 5:8:2, 1:ET - 1, :],
                              inT[:, 1:4:2, 2:ET, 2:2 + W],
                              inT[:, 1:4:2, 2:ET, 0:W])

            def stage_A_dy(s, prev=None):
                inm, inT = s["inm"], s["inT"]
                dym_t = wk.tile([128, 4 * E * W], f16, tag="dym")
                dym = dym_t.rearrange("p (s z x) -> p s z x", s=4, z=E)
                dyt_t = wk.tile([128, 4 * ET * W], f16, tag="dyt")
                dyt = dyt_t.rearrange("p (s z x) -> p s z x", s=4, z=ET)
                s["dym"], s["dyt"] = dym, dyt
                # main dym locals 0,1 (globals cZC-1, cZC) equal the previous
                # chunk's locals 6,7: one 4x-mode copy replaces 2 planes of
                # dy matmuls + drains. (dym is never clobbered by B2, unlike
                # dm, so the source is intact.)
                lp0 = 0
                if prev is not None:
                    TT.tensor_copy(out=dym[:, :, 0:2, :],
                                   in_=prev["dys"][:, :, :, :])
                    lp0 = 2

                def a_drain(sel, dst, pa):
                    src = pa.rearrange("p (a x) -> p a x", a=2)[:, :, 0:388] \
                        .rearrange("p a (f x) -> p a f x", f=2)
                    dstv = dst.rearrange("p (a f) x -> p a f x", a=2)
                    if sel:
                        TT.tensor_copy(out=dstv, in_=src)
                    else:
                        nc.scalar.copy(dstv, src)

                for lp in range(lp0, E):
                    pa = psA.tile([128, 1024], f32, tag="pa")
                    if lp < HC + 2:
                        kth, klt = 0, lp + 1
                        wkt = wk1m0b0[:, :]
                    else:
                        kth, klt = 64, lp - HC + 1
                        wkt = wk1m0b64[64:128, :]
                    for g in range(2):
                        pv = pa[:, 512 * g:512 * g + 388].rearrange(
                            "p (f x) -> p f x", f=2)
                        nc.tensor.matmul(pv[:, :, :], wk0m0[:, :],
                                         inm[:, 2 * g:2 * g + 2, lp + 1,
                                             1:1 + W],
                                         start=True, stop=False,
                                         skip_group_check=True)
                        if g == 0:
                            # dym slot 0 accumulates dz u1 too -> pre01
                            nc.tensor.matmul(pv[:, 0, :], wi[:, :],
                                             inm[:, 1, lp + 2, 1:1 + W],
                                             start=False, stop=False,
                                             skip_group_check=True)
                            nc.tensor.matmul(pv[:, 0, :], wim[:, :],
                                             inm[:, 1, lp, 1:1 + W],
                                             start=False, stop=False,
                                             skip_group_check=True)
                        nc.tensor.matmul(pv[:, :, :], wkt,
                                         inT[kth:kth + 64, 2 * g:2 * g + 2,
                                             klt, 1:1 + W],
                                         start=False, stop=True,
                                         skip_group_check=True)
                    a_drain(0, dym[:, 0:4, lp, :], pa)
                for lq in range(ET):
                    pa = psA.tile([128, 1024], f32, tag="pa")
                    for g in range(2):
                        for h, base in ((0, 0), (1, 64)):
                            lmain = lq + 1 + h * HC
                            pv = pa[base:base + 64,
                                    512 * g:512 * g + 388].rearrange(
                                "p (f x) -> p f x", f=2)
                            nc.tensor.matmul(pv[:, :, :], wk0m1[:, :],
                                             inm[:, 2 * g:2 * g + 2, lmain,
                                                 1:1 + W],
                                             start=True, stop=False,
                                             skip_group_check=True)
                        # both halves' tail-body Dy in one block-diag mm
                        pvf = pa[:, 512 * g:512 * g + 388].rearrange(
                            "p (f x) -> p f x", f=2)
                        nc.tensor.matmul(pvf[:, :, :], wk1m1bd[:, :],
                                         inT[:, 2 * g:2 * g + 2, lq + 1,
                                             1:1 + W],
                                         start=False, stop=True,
                                         skip_group_check=True)
                    a_drain(0, dyt[:, 0:4, lq, :], pa)

            def _b1_views(s):
                return ((s["dm"], s["dym"], s["fm"], s["dvm"], s["mom"],
                         s["inm"], E),
                        (s["dt"], s["dyt"], s["ft"], s["dvt"], s["mot"],
                         s["inT"], ET))

            def stage_B1_early(s):
                # 9-slot symmetric flux layout:
                # [0:t00 1:t01 2:t11 3:t02 4:ez 5:t12 6:ey 7:t22 8:ex]
                # full-E slots: 0,1,3,4 (z-streams); interior: 2,5,6,7,8
                # This part consumes ALL dym/dyt reads so next-chunk dy
                # matmuls+drains can start right after it.
                fm_t = wk.tile([128, 9 * E * W], f16, tag="fm")
                s["fm"] = fm_t.rearrange("p (s z x) -> p s z x", s=9, z=E)
                ft_t = wk.tile([128, 9 * ET * W], f16, tag="ft")
                s["ft"] = ft_t.rearrange("p (s z x) -> p s z x", s=9, z=ET)
                dvm_t = wk.tile([128, E * W], f16, tag="dv")
                s["dvm"] = dvm_t.rearrange("p (z x) -> p z x", z=E)
                dvt_t = wk.tile([128, ET * W], f16, tag="dvt")
                s["dvt"] = dvt_t.rearrange("p (z x) -> p z x", z=ET)
                mom_t = wk.tile([128, E * W], f16, tag="mo")
                s["mom"] = mom_t.rearrange("p (z x) -> p z x", z=E)
                mot_t = wk.tile([128, ET * W], f16, tag="mot")
                s["mot"] = mot_t.rearrange("p (z x) -> p z x", z=ET)
                for (d, dy, f, dv, mo, inp, ne) in _b1_views(s):
                    m2 = inp[:, 4, 1:1 + ne, 1:1 + W]
                    lo, hi = 1, ne - 1
                    m2i = m2[:, lo:hi, :]
                    TP.tensor_add(dv[:, :, :], d[:, 0, :, :], dy[:, 1, :, :])
                    TP.tensor_add(dv[:, :, :], dv[:, :, :], d[:, 6, :, :])
                    TT.tensor_scalar_mul(dv[:, :, :], dv[:, :, :], 1.0 / 3.0)
                    dvi = dv[:, lo:hi, :]
                    TT.tensor_sub(f[:, 2, lo:hi, :], dy[:, 1, lo:hi, :], dvi)
                    if ne == E:
                        # dym0 already holds pre01 = dy u0 + dz u1
                        TP.tensor_mul(f[:, 1, :, :], dy[:, 0, :, :], m2)
                    else:
                        TP.tensor_add(f[:, 1, :, :], d[:, 1, :, :],
                                      dy[:, 0, :, :])
                    TP.tensor_add(f[:, 5, lo:hi, :], dy[:, 2, lo:hi, :],
                                  d[:, 5, lo:hi, :])
                    TT.tensor_mul(f[:, 6, lo:hi, :], dy[:, 3, lo:hi, :], m2i)
                # stage main dym planes ZC,ZC+1 (globals (c+1)ZC-1,(c+1)ZC)
                # for the next chunk's copy: two-hop avoids a same-tag
                # read-old/write-new deadlock on the dym buffer.
                dys_t = wk.tile([128, 4 * 2 * W], f16, tag="dys")
                s["dys"] = dys_t.rearrange("p (s z x) -> p s z x", s=4, z=2)
                TT.tensor_copy(out=s["dys"][:, :, :, :],
                               in_=s["dym"][:, :, ZC:ZC + 2, :])

            def stage_B1_rest(s):
                for (d, dy, f, dv, mo, inp, ne) in _b1_views(s):
                    m2 = inp[:, 4, 1:1 + ne, 1:1 + W]
                    lo, hi = 1, ne - 1
                    m2i = m2[:, lo:hi, :]
                    dvi = dv[:, lo:hi, :]
                    # mo here is md = 2*m2: diag tau with u' = u/2 staging
                    TT.tensor_scalar_mul(mo[:, :, :], m2, 2.0)
                    # diag pre: t00 full E; t22 interior (t11 done in early)
                    TT.tensor_sub(f[:, 0, :, :], d[:, 0, :, :], dv[:, :, :])
                    TT.tensor_sub(f[:, 7, lo:hi, :], d[:, 6, lo:hi, :], dvi)
                    TP.tensor_add(f[:, 3, :, :], d[:, 2, :, :], d[:, 4, :, :])
                    # tau muls: diag x md(=2*m2), off-diag x m2; e-seeds x m2
                    TT.tensor_mul(f[:, 0, :, :], f[:, 0, :, :],
                                  mo[:, :, :])
                    TP.tensor_mul(f[:, 2:8:5, lo:hi, :],
                                  f[:, 2:8:5, lo:hi, :],
                                  mo[:, lo:hi, :].unsqueeze(1).broadcast_to(
                                      (128, 2, hi - lo, W)))
                    if ne == E:
                        TP.tensor_mul(f[:, 3, :, :], f[:, 3, :, :], m2)
                    else:
                        TP.tensor_mul(f[:, 1:4:2, :, :], f[:, 1:4:2, :, :],
                                      m2.unsqueeze(1).broadcast_to(
                                          (128, 2, ne, W)))
                    TT.tensor_mul(f[:, 5, lo:hi, :], f[:, 5, lo:hi, :],
                                  m2i)
                    TT.tensor_mul(f[:, 4, :, :], d[:, 3, :, :], m2)
                    TP.tensor_mul(f[:, 8, lo:hi, :], d[:, 7, lo:hi, :], m2i)

            def stage_B2(s, part=None):
                # e_j += sum_i tau_ij * u'_i with symmetric tau reuse.
                # ez(4) += t00*u0 + t01*u1 + t02*u2   (full E)
                # ey(6) += t01*u0 + t11*u1 + t12*u2   (interior)
                # ex(8) += t02*u0 + t12*u1 + t22*u2   (interior)
                # products staged in dead d-slots, grouped per u_i so the
                # three accumulate-adds are stride-2 f-slices.
                # part='a'/'b' splits the z-range (last chunk: lets C1-part-a
                # on PE overlap B2-part-b on DVE/Pool).
                for (d, f, dv, inp, ne) in (
                        (s["dm"], s["fm"], s["dvm"], s["inm"], E),
                        (s["dt"], s["ft"], s["dvt"], s["inT"], ET)):
                    mid = 5 if ne == E else 3
                    if part == "a":
                        lo, hi = 1, mid
                    elif part == "b":
                        lo, hi = mid, ne - 1
                    else:
                        lo, hi = 1, ne - 1
                    nei = hi - lo
                    ubi = [inp[:, i, 1 + lo:1 + hi, 1:1 + W] for i in range(3)]
                    ub2 = [u.unsqueeze(1).broadcast_to((128, 2, nei, W))
                           for u in ubi]
                    ub3 = [u.unsqueeze(1).broadcast_to((128, 3, nei, W))
                           for u in ubi]
                    ev = f[:, 4:9:2, lo:hi, :]
                    # u0 products: (t00,t01)->d[0:2], t02->d[2]
                    TP.tensor_mul(d[:, 0:2, lo:hi, :],
                                  f[:, 0:2, lo:hi, :], ub2[0])
                    TT.tensor_mul(d[:, 2, lo:hi, :],
                                  f[:, 3, lo:hi, :], ubi[0])
                    # u1 products: (t01,t11)->d[3:5], t12->d[5]
                    TT.tensor_mul(d[:, 3:5, lo:hi, :],
                                  f[:, 1:3, lo:hi, :], ub2[1])
                    TP.tensor_mul(d[:, 5, lo:hi, :],
                                  f[:, 5, lo:hi, :], ubi[1])
                    # u2 products: (t02,t12,t22)->d[6:9]
                    TT.tensor_mul(d[:, 6:9, lo:hi, :],
                                  f[:, 3:8:2, lo:hi, :], ub3[2])
                    TT.tensor_add(ev, ev, d[:, 0:3, lo:hi, :])
                    TP.tensor_add(ev, ev, d[:, 3:6, lo:hi, :])
                    TT.tensor_add(ev, ev, d[:, 6:9, lo:hi, :])
                    # edge planes (locals 0, ne-1): only ez consumed there
                    if part == "a":
                        ez_sl, ue_sl, npl = slice(0, 1), slice(1, 2), 1
                    elif part == "b":
                        ez_sl, ue_sl, npl = slice(ne - 1, ne), \
                            slice(ne, ne + 1), 1
                    else:
                        zs = ne - 1
                        ez_sl, ue_sl, npl = slice(0, ne, zs), \
                            slice(1, ne + 1, zs), 2
                    eze = f[:, 4, ez_sl, :]
                    for i, sl in ((0, 0), (1, 1), (2, 3)):
                        ue = inp[:, i, ue_sl, 1:1 + W]
                        TP.tensor_mul(dv[:, 0:npl, :], f[:, sl, ez_sl, :], ue)
                        TP.tensor_add(eze, eze, dv[:, 0:npl, :])

            def stage_C(s, g, mk=None, tk=None):
                # pass g=0 -> fields (mom0,mom1): z(0,1) y(1,2) x 3|5 singles
                # pass g=1 -> (mom2,energy):      z(3,4) y(5,6) x(7,8) pair
                # mk/tk: optional (k0,k1) plane subranges for main/tail
                mk = mk or (0, ZC)
                tk = tk or (0, HC)
                fm, ft = s["fm"], s["ft"]
                if g == 0:
                    om_t = wk.tile([128, 4 * ZC * N], f16, tag="om")
                    s["om"] = om_t.rearrange("p (f z x) -> p f z x",
                                             f=4, z=ZC)
                    ot_t = wk.tile([128, 4 * HC * N], f16, tag="ot")
                    s["ot"] = ot_t.rearrange("p (f z x) -> p f z x",
                                             f=4, z=HC)
                om, ot = s["om"], s["ot"]
                q = 2 * g
                z0 = 0 if g == 0 else 3
                y0 = 1 if g == 0 else 5

                def xmms(pv, f, le):
                    if g == 1:
                        nc.tensor.matmul(pv[:, :, :], wi[:, :],
                                         f[:, 7:9, le, 2:194],
                                         start=False, stop=False,
                                         skip_group_check=True)
                        nc.tensor.matmul(pv[:, :, :], wim[:, :],
                                         f[:, 7:9, le, 0:192],
                                         start=False, stop=False,
                                         skip_group_check=True)
                    else:
                        for fi, sl in ((0, 3), (1, 5)):
                            nc.tensor.matmul(pv[:, fi, :], wi[:, :],
                                             f[:, sl, le, 2:194],
                                             start=False, stop=False,
                                             skip_group_check=True)
                            nc.tensor.matmul(pv[:, fi, :], wim[:, :],
                                             f[:, sl, le, 0:192],
                                             start=False, stop=False,
                                             skip_group_check=True)

                for k in range(*mk):
                    le = k + 1
                    pc = psC.tile([128, 512], f32, tag="pc")
                    pv = pc[:, 0:384].rearrange("p (f x) -> p f x", f=2)
                    if k < HC:
                        kth, klt = 0, k + 1
                        wkt = wk1m0b0[:, :]
                    else:
                        kth, klt = 64, k - HC + 1
                        wkt = wk1m0b64[64:128, :]
                    nc.tensor.matmul(pv[:, :, :], wi[:, :],
                                     fm[:, z0:z0 + 2, le + 1, 1:193],
                                     start=True, stop=False,
                                     skip_group_check=True)
                    nc.tensor.matmul(pv[:, :, :], wim[:, :],
                                     fm[:, z0:z0 + 2, le - 1, 1:193],
                                     start=False, stop=False,
                                     skip_group_check=True)
                    xmms(pv, fm, le)
                    nc.tensor.matmul(pv[:, :, :], wk0m0[:, :],
                                     fm[:, y0:y0 + 2, le, 1:193],
                                     start=False, stop=False,
                                     skip_group_check=True)
                    nc.tensor.matmul(pv[:, :, :], wkt,
                                     ft[kth:kth + 64, y0:y0 + 2, klt,
                                        1:193],
                                     start=False, stop=True,
                                     skip_group_check=True)
                    nc.scalar.copy(om[:, q:q + 2, k, :], pv[:, :, :])
                for k in range(*tk):
                    le = k + 1
                    pc = psC.tile([128, 512], f32, tag="pc")
                    pv = pc[:, 0:384].rearrange("p (f x) -> p f x", f=2)
                    nc.tensor.matmul(pv[:, :, :], wi[:, :],
                                     ft[:, z0:z0 + 2, le + 1, 1:193],
                                     start=True, stop=False,
                                     skip_group_check=True)
                    nc.tensor.matmul(pv[:, :, :], wim[:, :],
                                     ft[:, z0:z0 + 2, le - 1, 1:193],
                                     start=False, stop=False,
                                     skip_group_check=True)
                    xmms(pv, ft, le)
                    for h, base in ((0, 0), (1, 64)):
                        pvh = pc[base:base + 64, 0:384].rearrange(
                            "p (f x) -> p f x", f=2)
                        nc.tensor.matmul(pvh[:, :, :], wk0m1[:, :],
                                         fm[:, y0:y0 + 2,
                                            k + 1 + h * HC, 1:193],
                                         start=False, stop=False,
                                         skip_group_check=True)
                    nc.tensor.matmul(pv[:, :, :], wk1m1bd[:, :],
                                     ft[:, y0:y0 + 2, le, 1:193],
                                     start=False, stop=True,
                                     skip_group_check=True)
                    nc.scalar.copy(ot[:, q:q + 2, k, :], pv[:, :, :])

            def stage_store(s):
                # outputs ride the Act DMA queue so they don't serialize
                # behind the next chunk's input loads on SP
                z0 = s["z0"]
                nc.scalar.dma_start(out=omain.ap()[:, :, z0:z0 + ZC, :],
                                    in_=s["om"][:, :, :, :])
                nc.scalar.dma_start(out=otail.ap()[:, :, z0:z0 + HC, :],
                                    in_=s["ot"][0:64, :, :, :])
                nc.scalar.dma_start(out=otail.ap()[:, :, z0 + HC:z0 + ZC, :],
                                    in_=s["ot"][64:128, :, :, :])

            # software-pipelined emission: A(c+1) is emitted between B2(c)
            # and C-g1(c) so PE/ACT fill B2's shadow and B1(c+1) can start
            # during C-g1(c)
            st = stage_load(0, first=True)
            load_weights()
            stage_A_dy(st)
            stage_A_sub(st)
            for c in range(nchunks):
                nxt = stage_load(c + 1) if c + 1 < nchunks else None
                stage_B1_early(st)
                if nxt is not None:
                    stage_A_dy(nxt, st)
                stage_B1_rest(st)
                stage_C(st, 0)
                if nxt is not None:
                    stage_B2(st)
                    stage_A_sub(nxt)
                    stage_C(st, 1)
                else:
                    # last chunk: split B2/C1 so C1-part-a (PE) overlaps
                    # B2-part-b (DVE/Pool) instead of a serial tail
                    stage_B2(st, "a")
                    stage_C(st, 1, mk=(0, 3), tk=(0, 1))
                    stage_B2(st, "b")
                    stage_C(st, 1, mk=(3, ZC), tk=(1, HC))
                stage_store(st)
                st = nxt

    nc.compile()
    return nc


_NC_CACHE = None


def _get_nc():
    global _NC_CACHE
    if _NC_CACHE is None:
        _NC_CACHE = build_program()
    return _NC_CACHE


def make_wts() -> np.ndarray:
    dy = np.zeros((N, N), dtype=np.float32)
    for m in range(N):
        dy[m, (m + 1) % N] = 1.0
        dy[m, (m - 1) % N] = -1.0
    dyt = dy.T
    eye = np.eye(128, dtype=np.float32)
    w = np.zeros((192, 576), dtype=np.float32)
    w[:, 0:192] = dyt
    w[0:128, 192:320] = eye
    w[0:128, 320:448] = -eye
    # block-diag tail-body Dy: both 64-row z-halves in one 128-wide matmul
    B = dyt[128:192, 128:192]
    w[0:64, 448:512] = B
    w[64:128, 512:576] = B
    return w.astype(np.float16)


def stage_fields(u, T):
    """Full-grid staged fields [5, N, N, WI] fp16 (x-halo'd, scaled)."""
    mu2 = (2.0 * MUSCALE) * np.power(T, 0.7, dtype=np.float32)
    tq = (CPR / 4.0) * T
    f = np.empty((5, N, N, N), dtype=np.float32)
    f[0:3] = 0.5 * u
    f[3] = tq
    f[4] = mu2
    fx = np.concatenate([f[..., -2:], f, f[..., :2]], axis=-1)
    return fx.astype(np.float16)


def shard_inputs(u, T, nz=NZ, ncores=NCORES):
    fx = stage_fields(u, T)          # [5, N, N, WI]
    wts = make_wts()
    in_maps = []
    for k in range(ncores):
        idx = np.arange(nz * k - 2, nz * k + nz + 2) % N
        blk = fx[:, idx, :, :]                     # [5, nz+4, N, WI]
        blk = np.ascontiguousarray(blk.transpose(2, 0, 1, 3))  # [y,5,z,x]
        in_maps.append({
            "amain": blk[0:128],
            "atail": np.ascontiguousarray(blk[128:192]),
            "wts": wts,
        })
    return in_maps


def kernel(u: np.ndarray, T: np.ndarray) -> np.ndarray:
    from concourse.bass_utils import run_bass_kernel_spmd

    u = np.asarray(u, dtype=np.float32)
    T = np.asarray(T, dtype=np.float32)
    nc = _get_nc()
    in_maps = shard_inputs(u, T)
    res = run_bass_kernel_spmd(nc, in_maps, list(range(NCORES)))

    out = np.zeros((5, N, N, N), dtype=np.float32)
    for k in range(NCORES):
        r = res.results[k]
        o = np.concatenate([np.asarray(r["omain"], dtype=np.float32),
                            np.asarray(r["otail"], dtype=np.float32)],
                           axis=0)                  # [192y, 4, nz, x]
        o = o.transpose(1, 2, 0, 3)                 # [4, nz, y, x]
        out[1:5, NZ * k:NZ * k + NZ] = o
    out[4] *= 2.0
    return out

